# revision 14
# baseline (speedup 1.0000x reference)
"""Trainium2 Bass kernel for the MoE routing module (nn_MoE_53042846105633).

Strategy: dense expert-parallel across 8 NeuronCores. Core e computes
expert e's MLP over ALL tokens (top-k masked-dense math, identical to the
reference), weights its output by that expert's routing weight (0 for
tokens that didn't pick it in top-2), and the host sums the 8 partials.

Router precision: top-2 selection must match an fp32 reference, so the
router's first matmul runs as bf16x2 (hi/lo split: W_h@x_h + W_h@x_l +
W_l@x_h accumulated in fp32 PSUM) and the logit matmul in fp32 on PE.
Expert MLP runs in plain bf16 (fp32 accumulate).
"""

import sys

sys.path.insert(0, "/opt/trn_rl_repo")

import numpy as np
import ml_dtypes

BF16 = ml_dtypes.bfloat16

# Model dims (fixed for this problem)
B = 1024          # tokens
DIN = 3072        # input features
RHID = 128        # router hidden
E = 8             # experts = cores
EHID = 2048       # expert hidden
NCLS = 10         # classes
KC1 = DIN // 128  # 24 K-chunks for DIN contraction
KC2 = EHID // 128 # 16 K-chunks for EHID contraction
MT = B // 128     # 8 token tiles
NT = B // 512     # 2 N-tiles of 512 tokens

_PROGRAM = None
LAST_RESULTS = None


def _ensure_axon_profile_hook():
    """bass_utils' trace=True path imports antenv.axon_hooks, which this
    image lacks. Provide it (backed by libaxon_pjrt.so's NRT profile C API)
    so NTFF profiling works; degrade silently if unavailable."""
    import contextlib
    import ctypes
    import os
    import types

    try:
        from antenv.axon_hooks import get_axon_ntff_profile_hook  # noqa: F401
        return
    except ImportError:
        pass
    try:
        import antenv
    except ImportError:
        return

    state = {"hook": None}
    mod = types.ModuleType("antenv.axon_hooks")
    mod.set_axon_ntff_profile_hook = lambda h: state.__setitem__("hook", h)
    mod.get_axon_ntff_profile_hook = lambda: state["hook"]
    sys.modules["antenv.axon_hooks"] = mod
    antenv.axon_hooks = mod

    so_path = "/opt/axon/libaxon_pjrt.so"
    if not os.path.exists(so_path):
        return
    try:
        lib = ctypes.CDLL(so_path)
    except OSError:
        return
    if not hasattr(lib, "axon_start_nrt_profile"):
        return
    lib.axon_start_nrt_profile.argtypes = [
        ctypes.POINTER(ctypes.c_int64), ctypes.c_size_t]
    lib.axon_start_nrt_profile.restype = ctypes.c_int64
    lib.axon_stop_nrt_profile.argtypes = [ctypes.c_char_p]
    lib.axon_stop_nrt_profile.restype = ctypes.c_int64

    @contextlib.contextmanager
    def _hook(output_dir, device_ids):
        import jax

        jax.devices()
        if device_ids:
            ids = (ctypes.c_int64 * len(device_ids))(*device_ids)
            rc = lib.axon_start_nrt_profile(ids, len(device_ids))
        else:
            rc = lib.axon_start_nrt_profile(None, 0)
        if rc != 0:
            raise RuntimeError(f"axon_start_nrt_profile rc={rc}")
        try:
            yield
        finally:
            n = lib.axon_stop_nrt_profile(str(output_dir).encode())
            print(f"profile: {n} ntff file(s) -> {output_dir}",
                  file=sys.stderr)

    state["hook"] = _hook


def _build_program():
    import concourse.tile as tile
    from concourse import bacc, mybir

    f32 = mybir.dt.float32
    bf = mybir.dt.bfloat16
    AF = mybir.ActivationFunctionType
    ALU = mybir.AluOpType

    # Bacc (not raw Bass): its compile() pass splits multi-sem waits onto
    # EventSemaphore instructions (TRN2 allows 1 wait per instruction).
    nc = bacc.Bacc("TRN2", debug=False, num_devices=E)

    # ---- DRAM I/O ----------------------------------------------------------
    # x (hi bf16), layout [i, k, n]: element = xf[n, 128k + i]
    d_xh = nc.dram_tensor("xh", [128, KC1, B], bf, kind="ExternalInput")
    # this core's 128-token slice of x in hi/lo bf16 split (router input)
    d_xrh = nc.dram_tensor("xrh", [128, KC1, 128], bf, kind="ExternalInput")
    d_xrl = nc.dram_tensor("xrl", [128, KC1, 128], bf, kind="ExternalInput")
    # router W1 (hi/lo), layout [i, k, j]: element = rW1[128k + i, j]
    d_w1h = nc.dram_tensor("w1h", [128, KC1, RHID], bf, kind="ExternalInput")
    d_w1l = nc.dram_tensor("w1l", [128, KC1, RHID], bf, kind="ExternalInput")
    d_rw2 = nc.dram_tensor("rw2", [RHID, E], f32, kind="ExternalInput")
    d_rb1 = nc.dram_tensor("rb1", [RHID, 1], f32, kind="ExternalInput")
    d_rb2 = nc.dram_tensor("rb2", [1, E], f32, kind="ExternalInput")
    # expert weights for this core's expert
    # ew1 layout [m, i, (k j)]: element = eW1[e][128k + i, 128m + j]
    d_ew1 = nc.dram_tensor("ew1", [KC2, 128, DIN], bf, kind="ExternalInput")
    # ew2 layout [i, k2, c]: element = eW2[e][128*k2 + i, c]
    d_ew2 = nc.dram_tensor("ew2", [128, KC2, NCLS], bf, kind="ExternalInput")
    # eb1 layout [i, m]: element = eb1[e][128m + i]
    d_eb1 = nc.dram_tensor("eb1", [128, KC2], f32, kind="ExternalInput")
    d_eb2 = nc.dram_tensor("eb2", [1, NCLS], bf, kind="ExternalInput")
    # one-hot row for this core's expert, tiled to 128 partitions
    d_sel = nc.dram_tensor("sel", [128, E], f32, kind="ExternalInput")
    # weighted partial output (host sums over cores)
    d_out = nc.dram_tensor("out", [B, NCLS], f32, kind="ExternalOutput")

    with tile.TileContext(nc) as tc:
        with (
            tc.tile_pool(name="const", bufs=1) as cp,
            tc.tile_pool(name="wstream", bufs=6) as wp,
            tc.tile_pool(name="psum", bufs=1, space="PSUM") as pp,
            tc.tile_pool(name="outp", bufs=1) as op,
            tc.tile_pool(name="dram", bufs=1, space="DRAM") as dp,
        ):
            # ---- input DMA (emission order ~= DMA queue order) -------------
            # Router inputs first (small, ~2.5 MB): the sharded router runs
            # on PE inside the DMA ramp while ew1/xk stream in.
            xrht = cp.tile([128, KC1, 128], bf, tag="xrh", name="xrht")
            nc.sync.dma_start(xrht[:], d_xrh[:])
            xrlt = cp.tile([128, KC1, 128], bf, tag="xrl", name="xrlt")
            nc.sync.dma_start(xrlt[:], d_xrl[:])
            w1ht = cp.tile([128, KC1, RHID], bf, tag="w1h", name="w1ht")
            nc.sync.dma_start(w1ht[:], d_w1h[:])
            w1lt = cp.tile([128, KC1, RHID], bf, tag="w1l", name="w1lt")
            nc.sync.dma_start(w1lt[:], d_w1l[:])
            rw2t = cp.tile([RHID, E], f32, tag="rw2", name="rw2t")
            nc.sync.dma_start(rw2t[:], d_rw2[:])
            rb1t = cp.tile([RHID, 1], f32, tag="rb1", name="rb1t")
            nc.sync.dma_start(rb1t[:], d_rb1[:])
            rb2t = cp.tile([1, E], f32, tag="rb2", name="rb2t")
            nc.sync.dma_start(rb2t[:], d_rb2[:])
            selt = cp.tile([128, E], f32, tag="sel", name="selt")
            nc.sync.dma_start(selt[:], d_sel[:])

            # Expert path inputs.
            wts = {}

            def load_ew1(m):
                wt = wp.tile([128, DIN], bf, tag="ew1", name=f"ew1m{m}")
                nc.sync.dma_start(wt[:], d_ew1[m])
                wts[m] = wt

            load_ew1(0)
            xk = []
            for k in range(KC1):
                t = cp.tile([128, B], bf, tag=f"xk{k}", name=f"xk{k}")
                nc.sync.dma_start(t[:], d_xh[:, k, :])
                xk.append(t)
            for _m in range(1, 6):
                load_ew1(_m)
            eb1t = cp.tile([128, KC2], f32, tag="eb1", name="eb1t")
            nc.sync.dma_start(eb1t[:], d_eb1[:])
            ew2t = cp.tile([128, KC2, NCLS], bf, tag="ew2", name="ew2t")
            nc.sync.dma_start(ew2t[:], d_ew2[:])
            eb2t = cp.tile([1, NCLS], bf, tag="eb2", name="eb2t")
            nc.sync.dma_start(eb2t[:], d_eb2[:])

            ones_f = cp.tile([1, 128], f32, tag="ones_f", name="ones_f")
            nc.gpsimd.memset(ones_f[:], 1.0)
            ones_b = cp.tile([1, 128], bf, tag="ones_b", name="ones_b")
            nc.gpsimd.memset(ones_b[:], 1.0)

            # ---- sharded router: this core routes its own 128 tokens ------
            # rhT = relu(rW1.T @ x_slice + rb1) via bf16x2 (3 passes)
            psr = pp.tile([128, 128], f32, tag="pr", bufs=1, name="psr")
            passes = [(w1ht, xrht), (w1ht, xrlt), (w1lt, xrht)]
            for pi, (wt_, xs_) in enumerate(passes):
                for k in range(KC1):
                    nc.tensor.matmul(
                        psr[:],
                        wt_[:, k, :],
                        xs_[:, k, :],
                        start=(pi == 0 and k == 0),
                        stop=(pi == 2 and k == KC1 - 1),
                    )
            rh = op.tile([RHID, 128], f32, tag="rh", name="rh")
            nc.scalar.activation(rh[:], psr[:], AF.Relu, bias=rb1t[:, 0:1])

            # logits [128 tok, E] in fp32 on PE (+rb2 via ones-row matmul)
            pl = pp.tile([128, E], f32, tag="lg", bufs=1, name="pl")
            nc.tensor.matmul(pl[:], rh[:], rw2t[:], start=True, stop=False)
            nc.tensor.matmul(pl[:], ones_f[0:1, :], rb2t[0:1, :],
                             start=False, stop=True)

            # top-2 weights for ALL experts on this token slice:
            #   w_all = exp(lg - m1) * (lg >= t2) / (1 + exp(t2 - m1))
            lg = op.tile([128, E], f32, tag="lg_sb", name="lg")
            nc.scalar.copy(lg[:], pl[:])
            m1 = op.tile([128, 1], f32, tag="m1", name="m1")
            nc.vector.reduce_max(m1[:], lg[:], axis=mybir.AxisListType.X)
            nm1 = op.tile([128, 1], f32, tag="nm1", name="nm1")
            nc.vector.tensor_scalar_mul(nm1[:], m1[:], -1.0)
            ismax = op.tile([128, E], f32, tag="ismax", name="ismax")
            nc.vector.tensor_scalar(ismax[:], lg[:], m1[:], None, ALU.is_ge)
            nc.vector.tensor_scalar_mul(ismax[:], ismax[:], -1e30)
            nc.vector.tensor_add(ismax[:], ismax[:], lg[:])
            t2 = op.tile([128, 1], f32, tag="t2", name="t2")
            nc.vector.reduce_max(t2[:], ismax[:], axis=mybir.AxisListType.X)
            w_all = op.tile([128, E], f32, tag="w_all", name="w_all")
            nc.vector.tensor_scalar(w_all[:], lg[:], t2[:], None, ALU.is_ge)
            enum = op.tile([128, E], f32, tag="enum", name="enum")
            nc.scalar.activation(enum[:], lg[:], AF.Exp, bias=nm1[:, 0:1])
            den = op.tile([128, 1], f32, tag="den", name="den")
            nc.scalar.activation(den[:], t2[:], AF.Exp, bias=nm1[:, 0:1])
            nc.vector.tensor_scalar_add(den[:], den[:], 1.0)
            rden = op.tile([128, 1], f32, tag="rden", name="rden")
            nc.vector.reciprocal(rden[:], den[:])
            nc.vector.tensor_mul(w_all[:], w_all[:], enum[:])
            nc.vector.tensor_scalar(w_all[:], w_all[:], rden[:], None, ALU.mult)

            # AllGather routing weights: [128, E] per core -> [B, E]
            wsl_d = dp.tile([128, E], f32, tag="wsl", name="wsl_d")
            nc.sync.dma_start(wsl_d[:], w_all[:])
            wag_d = dp.tile([B, E], f32, tag="wag", name="wag_d")
            nc.gpsimd.collective_compute(
                "AllGather",
                mybir.AluOpType.bypass,
                replica_groups=[list(range(E))],
                ins=[wsl_d.opt()],
                outs=[wag_d.opt()],
            )
            wag_sb = cp.tile([128, MT, E], f32, tag="wag_sb", name="wag_sb")
            nc.sync.dma_start(
                wag_sb[:], wag_d.rearrange("(t p) e -> p t e", p=128))
            # wmy[:, mt] = routing weight of this core's expert per token
            wmy = cp.tile([128, MT], f32, tag="wmy", name="wmy")
            for mt in range(MT):
                wsel = op.tile([128, E], f32, tag="wsel", bufs=2,
                               name=f"wsel{mt}")
                nc.vector.tensor_mul(wsel[:], wag_sb[:, mt, :], selt[:])
                nc.vector.reduce_sum(
                    wmy[:, mt:mt + 1], wsel[:], axis=mybir.AxisListType.X)

            # ehT: relu(eW1.T @ x) in [hid, tok] layout, bf16
            ehT = cp.tile([128, KC2, B], bf, tag="ehT", name="ehT")

            # ---- expert matmul 1: ehT[m] = relu(eW1[:, m-tile].T @ x + b) --
            for m in range(KC2):
                wt = wts[m]
                if m + 6 < KC2:
                    load_ew1(m + 6)
                for n in range(NT):
                    ps = pp.tile([128, 512], f32, tag="mm1", bufs=4,
                                 name=f"ps1_{m}_{n}")
                    for k in range(KC1):
                        nc.tensor.matmul(
                            ps[:],
                            wt[:, k * 128:(k + 1) * 128],
                            xk[k][:, n * 512:(n + 1) * 512],
                            start=(k == 0),
                            stop=(k == KC1 - 1),
                        )
                    nc.scalar.activation(
                        ehT[:, m, n * 512:(n + 1) * 512], ps[:],
                        AF.Relu, bias=eb1t[:, m:m + 1],
                    )

            # ---- expert matmul 2 + combine --------------------------------
            for mt in range(MT):
                po = pp.tile([128, NCLS], f32, tag="po", bufs=2, name=f"po{mt}")
                for k2 in range(KC2):
                    nc.tensor.matmul(
                        po[:],
                        ehT[:, k2, mt * 128:(mt + 1) * 128],
                        ew2t[:, k2, :],
                        start=(k2 == 0),
                        stop=False,
                    )
                nc.tensor.matmul(
                    po[:], ones_b[0:1, :], eb2t[0:1, :],
                    start=False, stop=True,
                )
                osb = op.tile([128, NCLS], f32, tag="osb", bufs=3,
                              name=f"osb{mt}")
                nc.vector.tensor_scalar(
                    osb[:], po[:], wmy[:, mt:mt + 1], None, ALU.mult,
                )
                nc.sync.dma_start(d_out[mt * 128:(mt + 1) * 128, :], osb[:])

    return nc


def _get_program():
    global _PROGRAM
    if _PROGRAM is None:
        _PROGRAM = _build_program()
    return _PROGRAM


def _prep_inputs(x, rW1, rb1, rW2, rb2, eW1, eb1, eW2, eb2):
    """Host-side shard/layout prep. Returns in_maps for the 8 cores."""
    xf = np.ascontiguousarray(x.reshape(B, DIN), dtype=np.float32)
    # [i, k, n] layout with hi/lo bf16 split
    xt = xf.reshape(B, KC1, 128).transpose(2, 1, 0)
    xh = xt.astype(BF16)
    xl = (xt - xh.astype(np.float32)).astype(BF16)
    xh = np.ascontiguousarray(xh)

    w1 = np.asarray(rW1, np.float32).reshape(KC1, 128, RHID).transpose(1, 0, 2)
    w1h = w1.astype(BF16)
    w1l = (w1 - w1h.astype(np.float32)).astype(BF16)
    w1h = np.ascontiguousarray(w1h)
    w1l = np.ascontiguousarray(w1l)

    rw2 = np.ascontiguousarray(np.asarray(rW2, np.float32))
    rb1c = np.ascontiguousarray(np.asarray(rb1, np.float32).reshape(RHID, 1))
    rb2r = np.ascontiguousarray(np.asarray(rb2, np.float32).reshape(1, E))

    in_maps = []
    for e in range(E):
        ew1 = np.ascontiguousarray(
            np.asarray(eW1[e], np.float32)
            .reshape(KC1, 128, KC2, 128)
            .transpose(2, 1, 0, 3)
            .reshape(KC2, 128, DIN)
            .astype(BF16)
        )
        ew2 = np.ascontiguousarray(
            np.asarray(eW2[e], np.float32)
            .reshape(KC2, 128, NCLS)
            .transpose(1, 0, 2)
            .astype(BF16)
        )
        eb1t = np.ascontiguousarray(
            np.asarray(eb1[e], np.float32).reshape(KC2, 128).T
        )
        eb2r = np.ascontiguousarray(
            np.asarray(eb2[e], np.float32).reshape(1, NCLS).astype(BF16)
        )
        sel = np.zeros((128, E), np.float32)
        sel[:, e] = 1.0
        tok = slice(128 * e, 128 * (e + 1))
        in_maps.append({
            "xh": xh,
            "xrh": np.ascontiguousarray(xh[:, :, tok]),
            "xrl": np.ascontiguousarray(xl[:, :, tok]),
            "w1h": w1h, "w1l": w1l,
            "rw2": rw2, "rb1": rb1c, "rb2": rb2r,
            "ew1": ew1, "ew2": ew2, "eb1": eb1t, "eb2": eb2r,
            "sel": sel,
        })
    return in_maps


def kernel(x, rW1, rb1, rW2, rb2, eW1, eb1, eW2, eb2):
    global LAST_RESULTS
    _ensure_axon_profile_hook()
    from concourse.bass_utils import run_bass_kernel_spmd

    nc = _get_program()
    if not nc.is_finalized():
        # bass2jax serializes the module as-is; Bacc's lowering passes
        # (register alloc, wait splitting) only run in finalize().
        nc.finalize()
    in_maps = _prep_inputs(x, rW1, rb1, rW2, rb2, eW1, eb1, eW2, eb2)
    res = run_bass_kernel_spmd(nc, in_maps, core_ids=list(range(E)))
    LAST_RESULTS = res
    out = np.zeros((B, NCLS), np.float32)
    for r in res.results:
        out += np.asarray(r["out"], np.float32)
    return out
